# revision 1
# baseline (speedup 1.0000x reference)
"""Trainium2 Bass kernel for the CrossEntropyMap loss.

Math (per batch row b of y_hat[B=64, T=64, G=128, G]):
    lse_b  = logsumexp(y_hat[b].reshape(-1))            # over T*G*G = 1M classes
    pick_b = sum_t y_hat[b, t, xi[b,t], yi[b,t]]        # xi/yi = round(coords*G)
    loss   = mean_b(T * lse_b - pick_b)

Sharding: data-parallel over batch, 8 rows per NeuronCore (32 MiB/core).
Each core streams its 8 rows as 16 half-row [128, 4096] f32 tiles and runs a
single ACT pass per chunk: exp(x + C_SHIFT) with accum_out giving the
per-partition sums. Any constant shift is mathematically exact for logsumexp
(it only scales the partial sums); C_SHIFT=-16 keeps exp in range for |x| up
to ~100. Per row r:
    S[p, r]  = sum_f exp(x[p, f] + C_SHIFT)          (ACT, accum_out)
    sums8[r] = sum_p S[p, r]                         (PE matmul with ones)
    lse_b    = ln(sums8[r]) - C_SHIFT                (ACT Ln; shift folded on host)
The 512 picked logits are gathered with indirect DMAs (one per pick column:
the HW DGE consumes one offset per partition per transfer). Each core emits
one scalar partial; the host sums the 8 partials, divides by B and adds the
shift constant.

Engine placement: even chunks DMA on SP (HWDGE), odd chunks on GpSimd
(SWDGE) — both otherwise idle, so their buffer-wait stalls never block ACT,
which only runs the exp/ln stream.

An optional numerically-defensive variant (USE_MAX=True) computes a real
per-partition running max on DVE and uses it as the exp bias, with a
max-stabilized cross-partition combine via PE transpose; it is ~10-15us
slower and only needed if inputs stop being ~N(0,1).
"""

import sys

import numpy as np

try:
    import concourse.bacc as bacc
except ImportError:  # pragma: no cover - fallback for bare environments
    sys.path.insert(0, "/opt/trn_rl_repo")
    import concourse.bacc as bacc

import concourse.bass as bass
import concourse.tile as tile
from concourse import mybir
from concourse.bass_utils import run_bass_kernel_spmd
from concourse.masks import make_identity

B, T, G = 64, 64, 128
N_CORES = 8
ROWS = B // N_CORES            # 8 batch rows per core
ROW_ELEMS = T * G * G          # 1_048_576 classes per row
P = 128
F = ROW_ELEMS // P             # 8192 elements per partition per row
HALVES = 2                     # DMA/ACT chunks per row
FH = F // HALVES               # 4096 per chunk
N_PER_CORE = ROWS * ROW_ELEMS  # 8_388_608 elements per core shard
PICKS = ROWS * T               # 512 gathered logits per core
PICK_F = PICKS // P            # 4 per partition
C_SHIFT = -16.0                # constant exp bias (exact for logsumexp)

_f32 = mybir.dt.float32
_i32 = mybir.dt.int32
_EXP = mybir.ActivationFunctionType.Exp
_LN = mybir.ActivationFunctionType.Ln
_AXF = mybir.AxisListType.X
_MAX = mybir.AluOpType.max
_MIN = mybir.AluOpType.min
_ADD = mybir.AluOpType.add
_SUB = mybir.AluOpType.subtract
_MUL = mybir.AluOpType.mult

USE_MAX = False   # defensive per-partition-max variant (slower)
_compiled_nc = None

# Test hook: BassKernelResults of the last run.
LAST_RESULTS = None


def build_nc(use_max: bool = USE_MAX):
    nc = bacc.Bacc("TRN2", target_bir_lowering=False, debug=False)
    y = nc.dram_tensor("y", [N_PER_CORE, 1], _f32, kind="ExternalInput")
    idx = nc.dram_tensor("idx", [P, PICK_F], _i32, kind="ExternalInput")
    out = nc.dram_tensor("out", [1, 1], _f32, kind="ExternalOutput")

    # [ROWS, HALVES, 128, 4096] chunk view: partition p of chunk (r, h) holds
    # elements [r*1M + p*8192 + h*4096, +4096) — contiguous per partition.
    y_chunks = y.ap().rearrange(
        "(r p h f) o -> r h p (f o)", r=ROWS, p=P, h=HALVES
    )

    with tile.TileContext(nc) as tc:
        with (
            tc.tile_pool(name="xpool", bufs=10) as xpool,
            tc.tile_pool(name="escratch", bufs=1) as escratch,
            tc.tile_pool(name="small", bufs=1) as small,
            tc.tile_pool(name="psum", bufs=1, space="PSUM") as psum,
        ):
            ones = small.tile([P, 1], _f32)
            nc.vector.memset(ones[:], 1.0)
            cbias = small.tile([P, 1], _f32)
            nc.vector.memset(cbias[:], C_SHIFT)
            if use_max:
                ident = small.tile([P, P], _f32)
                make_identity(nc, ident[:])
            idx_sb = small.tile([P, PICK_F], _i32)
            nc.sync.dma_start(out=idx_sb[:], in_=idx.ap())

            # --- stream the 8 rows as 16 half-row chunks ---
            # s_h[p, c] = sum_f exp(x[c][p, f] + C_SHIFT) per chunk c.
            # Software-pipelined trace order: prefill `bufs` DMAs, then
            # interleave exp(c) with dma(c + bufs) so the ACT-ring dispatches
            # (even/odd chunks split across the SP and ACT HWDGE rings) are
            # emitted right after the exp that frees their buffer slot and
            # never stall ACT's compute stream on a buffer-wait.
            n_chunks = ROWS * HALVES
            prefill = 10
            s_h = small.tile([P, n_chunks], _f32)
            neg_mh = small.tile([P, n_chunks], _f32) if use_max else None
            neg_m = small.tile([P, ROWS], _f32) if use_max else None
            x_tiles = {}

            def issue_dma(c):
                xt = xpool.tile([P, FH], _f32, tag="x")
                # Parity split only for the prefilled chunks (ACT prefetches
                # its 5 with free buffers); later chunks all go to the SP
                # ring, which has drained by then — the ACT ring otherwise
                # lags and gates the final exps.
                eng = nc.sync if (c % 2 == 0 or c >= prefill) else nc.scalar
                cr, ch = divmod(c, HALVES)
                eng.dma_start(out=xt[:], in_=y_chunks[cr, ch])
                x_tiles[c] = xt

            for c in range(min(prefill, n_chunks)):
                issue_dma(c)
            for c in range(n_chunks):
                xt = x_tiles.pop(c)
                cr, ch = divmod(c, HALVES)
                et = escratch.tile([P, FH], _f32, tag="e")
                if use_max:
                    nc.vector.tensor_reduce(
                        out=neg_mh[:, c : c + 1], in_=xt[:], axis=_AXF,
                        op=_MAX, negate=True,
                    )
                    if ch == HALVES - 1:
                        nc.vector.tensor_tensor(
                            out=neg_m[:, cr : cr + 1],
                            in0=neg_mh[:, c - 1 : c],
                            in1=neg_mh[:, c : c + 1],
                            op=_MIN,
                        )
                    bias = neg_m[:, cr : cr + 1]
                else:
                    bias = cbias[:, 0:1]
                nc.scalar.activation(
                    out=et[:], in_=xt[:], func=_EXP, bias=bias, scale=1.0,
                    accum_out=s_h[:, c : c + 1],
                )
                if c + prefill < n_chunks:
                    issue_dma(c + prefill)

            # --- picked-logit gather (emitted after the chunk loop so the
            # GpSimd queue prioritizes chunk DMA descriptors) ---
            picked = small.tile([P, PICK_F], _f32)
            for j in range(PICK_F):
                nc.gpsimd.indirect_dma_start(
                    out=picked[:, j : j + 1],
                    out_offset=None,
                    in_=y.ap(),
                    in_offset=bass.IndirectOffsetOnAxis(
                        ap=idx_sb[:, j : j + 1], axis=0
                    ),
                )
            # negpick[p] = -sum_j picked[p, j]
            negpick = small.tile([P, 1], _f32)
            nc.vector.tensor_reduce(
                out=negpick[:], in_=picked[:], axis=_AXF, op=_ADD, negate=True
            )

            # per-row sums: S[p, r] = s_h[p, 2r] + s_h[p, 2r+1]
            s_pairs = s_h[:].rearrange("p (r h) -> p r h", h=HALVES)
            s_sum = small.tile([P, ROWS], _f32)
            nc.vector.tensor_tensor(
                out=s_sum[:], in0=s_pairs[:, :, 0], in1=s_pairs[:, :, 1], op=_ADD
            )

            lse_t = small.tile([ROWS, 1], _f32)
            if use_max:
                # V[p, r] = ln(S) + m; stable cross-partition logsumexp via
                # PE transpose.
                v = small.tile([P, ROWS], _f32)
                nc.scalar.activation(out=v[:], in_=s_sum[:], func=_LN)
                nc.vector.tensor_tensor(out=v[:], in0=v[:], in1=neg_m[:], op=_SUB)
                vt = psum.tile([ROWS, P], _f32, tag="vt")
                nc.tensor.transpose(vt[:], v[:], ident[:])
                neg_m2 = small.tile([ROWS, 1], _f32)
                nc.vector.tensor_reduce(
                    out=neg_m2[:], in_=vt[:], axis=_AXF, op=_MAX, negate=True
                )
                e2 = small.tile([ROWS, P], _f32)
                s2 = small.tile([ROWS, 1], _f32)
                nc.scalar.activation(
                    out=e2[:], in_=vt[:], func=_EXP, bias=neg_m2[:], scale=1.0,
                    accum_out=s2[:],
                )
                ln2 = small.tile([ROWS, 1], _f32)
                nc.scalar.activation(out=ln2[:], in_=s2[:], func=_LN)
                nc.vector.tensor_scalar(
                    out=lse_t[:], in0=ln2[:], scalar1=neg_m2[:, 0:1],
                    scalar2=float(T), op0=_SUB, op1=_MUL,
                )
            else:
                # sums8[r] = sum_p S[p, r] via PE; lse'_r = ln(sums8[r]).
                sums8 = psum.tile([ROWS, 1], _f32, tag="sums8")
                nc.tensor.matmul(
                    out=sums8[:], lhsT=s_sum[:], rhs=ones[:], start=True,
                    stop=True,
                )
                ln8 = small.tile([ROWS, 1], _f32)
                nc.scalar.activation(out=ln8[:], in_=sums8[:], func=_LN)
                nc.vector.tensor_scalar(
                    out=lse_t[:], in0=ln8[:], scalar1=float(T), scalar2=None,
                    op0=_MUL,
                )

            # partial = sum_p D[p], D = -picked_sums; D[0:ROWS] += T*lse'.
            nc.vector.tensor_tensor(
                out=negpick[0:ROWS, :], in0=negpick[0:ROWS, :], in1=lse_t[:],
                op=_ADD,
            )
            acc = psum.tile([1, 1], _f32, tag="acc")
            nc.tensor.matmul(
                out=acc[:], lhsT=negpick[:], rhs=ones[:], start=True, stop=True
            )
            res = small.tile([1, 1], _f32)
            nc.vector.tensor_copy(out=res[:], in_=acc[:])
            nc.sync.dma_start(out=out.ap(), in_=res[:])

    nc.compile()
    return nc


def make_in_maps(y_hat: np.ndarray, coords: np.ndarray):
    """Shard inputs across cores and build per-core gather indices."""
    y_hat = np.ascontiguousarray(y_hat, dtype=np.float32)
    coords = np.asarray(coords, dtype=np.float32)

    # Match jnp.round (round-half-to-even); np.round has identical semantics,
    # and coords * 128 is exact in f32 (power-of-two scale).
    xi = np.round(coords[:, :, 0] * np.float32(G)).astype(np.int64)  # (B, T)
    yi = np.round(coords[:, :, 1] * np.float32(G)).astype(np.int64)  # (B, T)
    t = np.arange(T, dtype=np.int64)[None, :]
    flat = t * (G * G) + xi * G + yi  # (B, T) element offset within row b

    in_maps = []
    for c in range(N_CORES):
        rows = slice(c * ROWS, (c + 1) * ROWS)
        shard = y_hat[rows].reshape(N_PER_CORE, 1)
        local = np.arange(ROWS, dtype=np.int64)[:, None] * ROW_ELEMS + flat[rows]
        idx = local.reshape(P, PICK_F).astype(np.int32)
        in_maps.append({"y": shard, "idx": idx})
    return in_maps


def kernel(y_hat: np.ndarray, coords: np.ndarray) -> np.ndarray:
    global _compiled_nc, LAST_RESULTS
    in_maps = make_in_maps(y_hat, coords)
    if _compiled_nc is None:
        _compiled_nc = build_nc()
    res = run_bass_kernel_spmd(
        _compiled_nc, in_maps, core_ids=list(range(N_CORES))
    )
    LAST_RESULTS = res
    total = 0.0
    for r in res.results:
        total += float(np.asarray(r["out"]).reshape(()))
    loss = total / B
    if not USE_MAX:
        loss += T * (-C_SHIFT)  # lse_b = lse'_b - C_SHIFT, folded over all rows
    return np.array(np.float32(loss))

